# revision 42
# baseline (speedup 1.0000x reference)
"""Multi-head attention (softmax over the QUERY axis) on 8 TRN2 NeuronCores.

Problem shapes: Q [T=1024, B=8, D=256]; per-head projections Wq/Wk/Wv
[H=8, E=512, D=256]; Wo [D=256, H*E=4096].

Sharding: data-parallel over batch B — core b computes all H heads for
batch b. No collectives; the host re-stacks per-core outputs along B.

Algebraic reduction (softmax is over the query axis t, so any additive
term that is constant in t cancels in the softmax):
  A[t,s] = (Wq x_t + bq)·(Wk x_s + bk)·s
         = x_t·(s·Wq^T Wk x_s + s·Wq^T bk) + [terms const in t -> dropped]
  => k~_s = M~ x_s + u~  with  M~ = s·(Wq^T Wk) [D,D],  u~ = s·Wq^T bk
  => A'[t,s] = x_t·k~_s   (D=256 contraction instead of E=512, and no
     separate q projection at all)
Similarly Wo folds into Wv:
  out[t,d] = sum_h sum_s attn_h[t,s]·v~_h[s,d] + bo
  v~_h[s]  = C_h x_s + c_h,  C_h = Wo_h Wv_h [D,D],  c_h = Wo_h bv_h
M~, u~, C, c are precomputed on the host (free). Device FLOPs drop from
25.8G to 10.7G per core.

The scores matmul runs in fp8 (e4m3) DoubleRow perf mode: k~ is scaled
by 32 into fp8 range, the Exp activation divides the 32 back out via
its scale operand. Everything else stays bf16 (fp8 there busts the 2e-2
error budget).

Software pipeline (PE program order), steady state for head h:
  for sb in 0..7:  scores(h,sb) -> exp on Scalar;  out(h-1, tb=sb)
                   2 proj(h+1) matmul groups
so the PE streams out-chains while Scalar exps — the at-tile pool never
throttles the PE.
"""

import sys

sys.path.insert(0, "/opt/trn_rl_repo")

from contextlib import ExitStack

import ml_dtypes
import numpy as np

import concourse.bass as bass
import concourse.tile as tile
from concourse.tile import add_dep_helper
from concourse import bacc, bass_utils, mybir

T, B, D, H, E = 1024, 8, 256, 8, 512
N_CORES = 8

F32 = mybir.dt.float32
BF16 = mybir.dt.bfloat16
F8 = mybir.dt.float8e4
AF = mybir.ActivationFunctionType
ALU = mybir.AluOpType
DR = mybir.MatmulPerfMode.DoubleRow
K8SCALE = 32.0


def _bcast(ap_row, parts):
    """Partition-broadcast a [1, n] DRAM AP to [parts, n] (step-0 partition)."""
    return bass.AP(
        tensor=ap_row.tensor,
        offset=ap_row.offset,
        ap=[[0, parts], list(ap_row.ap[-1])],
    )


def build_nc(t=T, d=D, h=H):
    """Build the per-core SPMD program. Returns a compiled Bacc."""
    DC = d // 128   # d chunks (contraction for everything)
    SB = t // 128   # s blocks (key-chunk partition tiles)
    TB = t // 128   # t blocks (partition tiles of out)
    TC = t // 512   # t chunks (512-wide matmul free dim)
    assert DC == 2, "fp8 DoubleRow path assumes a 256-deep contraction"

    nc = bacc.Bacc("TRN2", target_bir_lowering=False, debug=False)

    xt_d = nc.dram_tensor("xt", [128, DC, t], BF16, kind="ExternalInput").ap()
    mk_d = nc.dram_tensor("mk", [h, 128, DC, d], BF16, kind="ExternalInput").ap()
    ct_d = nc.dram_tensor("ct", [h, 128, DC, d], BF16, kind="ExternalInput").ap()
    u_d = nc.dram_tensor("us", [128, h, DC], F32, kind="ExternalInput").ap()
    c_d = nc.dram_tensor("cs", [h, d], F32, kind="ExternalInput").ap()
    bo_d = nc.dram_tensor("bo2", [d], F32, kind="ExternalInput").ap()
    out_d = nc.dram_tensor("out", [t, d], F32, kind="ExternalOutput").ap()

    with tile.TileContext(nc) as tc, ExitStack() as ctx:
        consts = ctx.enter_context(tc.tile_pool(name="consts", bufs=1))
        wpool = ctx.enter_context(tc.tile_pool(name="wpool", bufs=3))
        hpool = ctx.enter_context(tc.tile_pool(name="hpool", bufs=2))
        kvpool = ctx.enter_context(tc.tile_pool(name="kvpool", bufs=3))
        spool = ctx.enter_context(tc.tile_pool(name="spool", bufs=2))
        at_pool = ctx.enter_context(tc.tile_pool(name="at_pool", bufs=2, space="PSUM"))
        mm_pool = ctx.enter_context(tc.tile_pool(name="mm_pool", bufs=2, space="PSUM"))
        out_pool = ctx.enter_context(tc.tile_pool(name="out_pool", bufs=2, space="PSUM"))

        # ---- PE warm-up: dummy matmuls during the initial DMA wait keep the
        # PE clock ramp (HAM gate) going until real work lands; count tuned
        # so they end right as the first weights arrive -------------------
        scratch = consts.tile([128, 640], BF16)
        nc.gpsimd.memset(scratch, 0.0)
        ps_w = mm_pool.tile([128, 512], F32, tag="mm")
        first_mm = {}
        for _ in range(12):
            mm = nc.tensor.matmul(
                ps_w, scratch[:, :128], scratch[:, 128:640], start=True, stop=True
            )
            first_mm.setdefault("mm", mm)

        # ---- persistent tiles -------------------------------------------
        xt_sb = consts.tile([128, DC, t], BF16)
        xt8 = consts.tile([128, DC, t], F8)
        u_sb = consts.tile([128, h, DC], F32)
        out_acc = consts.tile([128, TB, d], F32)
        out_r = out_d.rearrange("(tb p) d -> p tb d", p=128)

        def load_head(hh):
            gated = []
            mk_sb = wpool.tile([128, DC, d], BF16, tag="mk")
            if hh == 0:
                # per-block halves: the first k~ group only needs cols 0:128
                nc.sync.dma_start(out=mk_sb[:, :, :128], in_=mk_d[hh, :, :, :128])
                nc.sync.dma_start(out=mk_sb[:, :, 128:], in_=mk_d[hh, :, :, 128:])
            else:
                gated.append(nc.sync.dma_start(out=mk_sb, in_=mk_d[hh]))
            ct_sb = wpool.tile([128, DC, d], BF16, tag="ct")
            gated.append(nc.gpsimd.dma_start(out=ct_sb, in_=ct_d[hh]))
            c_bc = wpool.tile([128, d], F32, tag="cb")
            nc.gpsimd.dma_start(out=c_bc, in_=_bcast(c_d[hh][None, :], 128))
            anchor = first_mm.get("xt") or first_mm.get("mm")
            if hh >= 1 and anchor is not None:
                # keep later heads' bulk loads out of the DMA queues until
                # head 0's critical transfers have landed (cold-start path)
                for g in gated:
                    add_dep_helper(g.ins, anchor.ins, reason="cold start")
            return mk_sb, ct_sb, c_bc

        def proj_pieces(hh, w):
            """Generator of proj matmul groups for head hh: k~8 [d,s] fp8
            (scaled x32, via fused DVE add+mul) and vF [s,d] f32."""
            mk_sb, ct_sb, c_bc = w
            kt8 = kvpool.tile([128, DC, t], F8, tag="kt8")
            vF = kvpool.tile([128, SB, d], F32, tag="vF")
            # heads >= 1 run the k~ projection in fp8 DoubleRow (M~ scaled x64
            # into fp8 range); head 0 stays bf16 — its operands sit on the
            # cold-start critical path before the fp8 casts are ready, and it
            # buys back a little accuracy. us holds 32*u~ so the PSUM->fp8
            # copy is (ps*imm + 32u~) with imm = 32 (bf16 ps) or 0.5 (x64 ps).
            mk8 = None
            if hh >= 1:
                mk8 = wpool.tile([128, DC, d], F8, tag="mk8")
            cast_done = [False]
            for blk in range(DC):
                for sch in range(TC):
                    ssl = slice(sch * 512, (sch + 1) * 512)

                    def k_group(blk=blk, ssl=ssl):
                        if mk8 is not None and not cast_done[0]:
                            nc.scalar.activation(mk8, mk_sb, AF.Copy, scale=64.0)
                            cast_done[0] = True
                        ps = mm_pool.tile([128, 512], F32, tag="mm")
                        if mk8 is None:
                            for dc in range(DC):
                                mm = nc.tensor.matmul(
                                    ps,
                                    mk_sb[:, dc, blk * 128 : (blk + 1) * 128],
                                    xt_sb[:, dc, ssl],
                                    start=(dc == 0),
                                    stop=(dc == DC - 1),
                                )
                                first_mm.setdefault("mm", mm)
                            imm = K8SCALE
                        else:
                            nc.tensor.matmul(
                                ps,
                                mk8[:, :, blk * 128 : (blk + 1) * 128],
                                xt8[:, :, ssl],
                                start=True,
                                stop=True,
                                perf_mode=DR,
                            )
                            imm = K8SCALE / 64.0
                        nc.vector.tensor_scalar(
                            kt8[:, blk, ssl],
                            ps,
                            imm,
                            u_sb[:, hh, blk : blk + 1],
                            ALU.mult,
                            ALU.add,
                        )

                    yield k_group
            for sb in range(SB):
                ssl = slice(sb * 128, (sb + 1) * 128)

                def v_group(sb=sb, ssl=ssl):
                    ps = mm_pool.tile([128, 512], F32, tag="mm")
                    for dc in range(DC):
                        nc.tensor.matmul(
                            ps[:, :d],
                            xt_sb[:, dc, ssl],
                            ct_sb[:, dc, :],
                            start=(dc == 0),
                            stop=(dc == DC - 1),
                        )
                    nc.vector.tensor_add(vF[:, sb, :], ps[:, :d], c_bc)

                yield v_group
            proj_out[hh] = (kt8, vF)

        def scores_step(hh, sb, kt8, vF, Ex, Vv, lsum, rr):
            """One s-block of scores (fp8 DoubleRow) + one wide exp + 1/l."""
            ssl = slice(sb * 128, (sb + 1) * 128)
            at = at_pool.tile([128, TC, 512], F32, tag="at")
            for tch in range(TC):
                tsl = slice(tch * 512, (tch + 1) * 512)
                nc.tensor.matmul(
                    at[:, tch, :],
                    kt8[:, :, ssl],
                    xt8[:, :, tsl],
                    start=True,
                    stop=True,
                    perf_mode=DR,
                )
            # single activation over both PSUM banks: 1 instr, 1 accum read
            nc.scalar.activation(
                Ex[:, sb, :],
                at,
                AF.Exp,
                scale=1.0 / K8SCALE,
                accum_out=lsum[:, sb : sb + 1],
            )

        def scores_tail(hh, sb, vF, Vv, lsum, rr):
            nc.vector.reciprocal(rr[:, sb : sb + 1], lsum[:, sb : sb + 1])
            nc.vector.tensor_scalar_mul(Vv[:, sb, :], vF[:, sb, :], rr[:, sb : sb + 1])

        def out_chain(hh, tb, Ex, Vv):
            """out_acc[t,d] += sum_s E[s,t]·v'[s,d] (PSUM chain over s)."""
            ps = out_pool.tile([128, 512], F32, tag="out")
            for sb in range(SB):
                nc.tensor.matmul(
                    ps[:, :d],
                    Ex[:, sb, tb * 128 : (tb + 1) * 128],
                    Vv[:, sb, :],
                    start=(sb == 0),
                    stop=(sb == SB - 1),
                )
            if hh == 0:
                nc.vector.tensor_add(out_acc[:, tb, :], ps[:, :d], bo_bc)
            else:
                nc.vector.tensor_add(out_acc[:, tb, :], out_acc[:, tb, :], ps[:, :d])
            if hh == h - 1:
                # rotate store issue across sequencers: each DMA issue costs
                # ~600ns on the issuing engine, serializing the drain if all
                # eight go through one queue
                eng = (nc.sync, nc.gpsimd, nc.scalar)[tb % 3]
                eng.dma_start(out=out_r[:, tb, :], in_=out_acc[:, tb, :])

        # ---- software pipeline over heads -------------------------------
        # head 0: proj(0) inline, then its scores interleave proj(1)+proj(2)
        # (there is no out(-1) work, so double up on proj to feed the PE
        # while Scalar exps). head 1 then has no proj; heads 2..6 do
        # proj(h+1); head 7 none. out(h-1) interleaves into head h; out(7)
        # drains at the end.
        proj_out = {}
        # xt is the big transfer everything depends on: issue it first
        nc.sync.dma_start(out=xt_sb[:, :, : t // 2], in_=xt_d[:, :, : t // 2])
        xt_dma = nc.sync.dma_start(out=xt_sb[:, :, t // 2 :], in_=xt_d[:, :, t // 2 :])
        # gate later heads' weight loads on the xt transfer finishing: they
        # then start with zero contention on the cold critical path, yet
        # still land in time for the proj pieces interleaved into head 0
        first_mm["xt"] = xt_dma
        nc.scalar.dma_start(out=u_sb, in_=u_d)
        w = load_head(0)
        bo_bc = consts.tile([128, d], F32)
        nc.gpsimd.dma_start(out=bo_bc, in_=_bcast(bo_d[None, :], 128))
        for dc in range(DC):
            nc.vector.tensor_copy(xt8[:, dc, :], xt_sb[:, dc, :])
        for g in proj_pieces(0, w):   # head 0 projections run un-interleaved
            g()
        ex_tiles = {}
        for hh in range(h):
            if hh == 0:
                pieces = list(proj_pieces(1, load_head(1)))
                pieces += list(proj_pieces(2, load_head(2)))
            elif 2 <= hh < h - 1:
                pieces = list(proj_pieces(hh + 1, load_head(hh + 1)))
            else:
                pieces = []
            kt8, vF = proj_out[hh]
            Ex = hpool.tile([128, SB, t], BF16, tag="Ex")
            Vv = hpool.tile([128, SB, d], BF16, tag="Vv")
            lsum = spool.tile([128, SB], F32, tag="l")
            rr = spool.tile([128, SB], F32, tag="rr")
            ex_tiles[hh] = (Ex, Vv)
            pi = 0
            npc = len(pieces)
            for sb in range(SB):
                scores_step(hh, sb, kt8, vF, Ex, Vv, lsum, rr)
                if hh >= 1:
                    out_chain(hh - 1, sb, *ex_tiles[hh - 1])
                # spread the proj groups of later heads across the 8 steps
                n_here = (npc * (sb + 1) + 7) // 8 - (npc * sb + 7) // 8
                for _ in range(n_here):
                    if pi < npc:
                        pieces[pi]()
                        pi += 1
                scores_tail(hh, sb, vF, Vv, lsum, rr)
            while pi < npc:
                pieces[pi]()
                pi += 1
        # drain: last head's out stage
        for tb in range(TB):
            out_chain(h - 1, tb, *ex_tiles[h - 1])

    nc.compile()
    return nc


_NC_CACHE = {}


def _get_nc(shape_key):
    if shape_key not in _NC_CACHE:
        _NC_CACHE[shape_key] = build_nc(*shape_key)
    return _NC_CACHE[shape_key]


def _pmajor(a, last):
    """[..., C*128, last] -> [..., 128, C, last] partition-major layout."""
    lead = a.shape[:-2]
    c = a.shape[-2] // 128
    return np.ascontiguousarray(
        a.reshape(*lead, c, 128, last).swapaxes(-3, -2)
    )


def _prep_inputs(Q, Wq, bq, Wk, bk, Wv, bv, Wo, bo):
    t, b, d = Q.shape
    h, e, _ = Wq.shape
    s = np.float32(1.0 / np.sqrt(e))
    bf = ml_dtypes.bfloat16
    Q = np.asarray(Q, np.float32)
    Wq = np.asarray(Wq, np.float32)
    Wk = np.asarray(Wk, np.float32)
    Wv = np.asarray(Wv, np.float32)
    Wo = np.asarray(Wo, np.float32)
    bk = np.asarray(bk, np.float32)
    bv = np.asarray(bv, np.float32)

    # Fused weights (host-side, fp32): bq cancels in the query-axis softmax.
    M = np.einsum("hed,hef->hdf", Wq, Wk) * s          # [H,D,D]  A-term x_t M x_s
    u = np.einsum("hed,he->hd", Wq, bk) * s            # [H,D]
    WoR = Wo.reshape(d, h, e)
    C = np.einsum("dhe,hef->hdf", WoR, Wv)             # [H,D,D]  v~ = C x + c
    c = np.einsum("dhe,he->hd", WoR, bv)               # [H,D]

    # matmul operand layouts: mk[h, dc*128+p, d'] = M[h, d', d];
    # ct[h, dc*128+p, d'] = C[h, d', d]  (both transposed, partition-major)
    mk = _pmajor(np.ascontiguousarray(M.transpose(0, 2, 1)).astype(bf), d)
    ct = _pmajor(np.ascontiguousarray(C.transpose(0, 2, 1)).astype(bf), d)
    qt_all = _pmajor(Q.transpose(1, 2, 0).astype(bf), t)   # [B,128,DC,T]

    shared = {
        "mk": mk,
        "ct": ct,
        "us": np.ascontiguousarray((32.0 * u).reshape(h, -1, 128).transpose(2, 0, 1)),
        "cs": np.ascontiguousarray(c),
        "bo2": np.ascontiguousarray(np.asarray(bo, np.float32)),
    }
    in_maps = [
        {"xt": np.ascontiguousarray(qt_all[bb]), **shared} for bb in range(b)
    ]
    return in_maps, (t, d, h)


def kernel(Q, Wq, bq, Wk, bk, Wv, bv, Wo, bo, _trace=False):
    in_maps, (t, d, h) = _prep_inputs(Q, Wq, bq, Wk, bk, Wv, bv, Wo, bo)
    nc = _get_nc((t, d, h))
    res = bass_utils.run_bass_kernel_spmd(
        nc, in_maps, core_ids=list(range(len(in_maps))), trace=_trace
    )
    out = np.stack([res.results[b]["out"] for b in range(len(in_maps))], axis=1)
    if _trace:
        kernel.last_results = res
    return np.ascontiguousarray(out.astype(np.float32))
